# revision 18
# baseline (speedup 1.0000x reference)
"""Inverse Hough transform (nn_C_iht) on 8 Trainium2 NeuronCores.

out[n,c,y,x] = sum_a hough[n,c,a, r(a,y,x)]  with a static index table r.

Strategy (per core; batch n is sharded across the 8 cores, c=128 channels sit
on the SBUF partition axis):
  - The gather-sum is evaluated as a sequence of one-hot matmuls on the
    TensorEngine.  For a pixel block P (8 wide x 16 tall = 128 pixels) and a
    chunk C = (16 consecutive angles) x (8 consecutive rhos), K = 128:
        psum[c, px] += Hp_chunk[k, c].T  @  E_chunk[k, px]
    where Hp_chunk is an affine slice of a host-side rectangle re-layout of
    the input and E_chunk in {0,1} is the (static) one-hot selector
    E[(ai,rj), px] = [ r(a, px) == rho ].
  - E is generated on-chip on the Vector engine from a streamed int16 index
    table D[p,px] = r(a(p),px) - 8*rlo.  Crucially E-gen uses
    nc.vector.tensor_scalar (InstTensorScalarPtr, is_stt=False), the only
    elementwise op family that supports the DVE 4x_2p mode (4 elem/cyc/lane);
    scalar_tensor_tensor runs at 1x and was the previous bottleneck (1.06ms).
    Layout is plane-major per slab: tiles sorted by their chunk count nch,
    so chunk-plane n covers a contiguous suffix of tiles and is generated by
    ONE op:  E_n = (D - 8n) == rj   (rj = per-partition scalar, 8n = imm).
  - D has only 16 distinct rows (the 8 rho-subrows are replicates), so the
    DMA streams a [16, .] table and replicates partitions via a stride-0
    source dim: 78.6MB -> 9.8MB of HBM traffic.
  - PSUM drains run on the Scalar (Act) engine to keep the DVE free.
"""

import sys

sys.path.insert(0, "/opt/trn_rl_repo")

import numpy as np
import ml_dtypes

N, C, HIMG, WIMG = 8, 128, 160, 160
NUMANGLE, NUMRHO = 180, 180

# chunk geometry
G = 18         # angles per chunk
B = 7          # rhos per chunk  (G * B = 126 = contraction dim K)
K = G * B      # used contraction partitions (126)
APAD = 180     # angle count (divides G exactly -> no padded angle rows)
RPAD = 182     # padded rho count (26 rho-blocks of 7)
NG = APAD // G          # 12 angle groups
NR = RPAD // B          # 24 rho blocks
BW, BH = 8, 16          # pixel block: 8 wide (x), 16 tall (y) -> 128 px
PX = BW * BH
NBX, NBY = WIMG // BW, HIMG // BH   # 20 x 10 = 200 blocks
XGRP = 2                # blocks per slab (output staging + E-gen batch)
TPS = XGRP * NG         # D tiles per slab (24)
NSLAB = NBY * (NBX // XGRP)

BF16 = ml_dtypes.bfloat16
PAD_D = 20000           # D value for padded angle rows (never matches rj)


def _rho_table() -> np.ndarray:
    """Exact replica of the reference's index table r[a, y, x]."""
    irho = (int(np.sqrt(HIMG * HIMG + WIMG * WIMG)) + 1) / float(NUMRHO)
    itheta = np.pi / NUMANGLE
    theta = np.arange(NUMANGLE) * itheta
    tab_cos = np.cos(theta) / irho
    tab_sin = np.sin(theta) / irho
    xs = np.arange(WIMG) - WIMG // 2
    ys = np.arange(HIMG) - HIMG // 2
    r = np.round(xs[None, None, :] * tab_cos[:, None, None]
                 + ys[None, :, None] * tab_sin[:, None, None]).astype(np.int64)
    return np.clip(r + NUMRHO // 2, 0, NUMRHO - 1)  # [A, H, W]


def _build_schedule():
    """Static slab schedule + packed int16 D stream (16 rows per tile).

    Returns (slabs, d_stream, nch_max, emax):
      slabs: list (by-major, then bx-slab) of dicts with
        'planes': [(first_tile, e_base)] for plane n = 0..smax-1
        'blocks': [[(g, rlo, nch, ecols) x NG] x XGRP] consumption info,
          ecols[k] = E column index (in PX units) of chunk k.
      d_stream: [G, NSLAB*TPS*PX] int16, one [G, PX] tile per slab entry in
        nch-sorted order (row ai holds r(a0+ai, px) - 8*rlo).
    """
    R = _rho_table()
    slabs = []
    d_parts = []
    nch_max = 0
    emax = 0
    for by in range(NBY):
        for bxg in range(NBX // XGRP):
            tiles = []  # (nch, bxi, g, rlo, D16)
            for bxi in range(XGRP):
                bx = bxg * XGRP + bxi
                sub = R[:, by * BH:(by + 1) * BH, bx * BW:(bx + 1) * BW]
                sub = sub.reshape(NUMANGLE, PX)  # px = dy*BW + dx
                for g in range(NG):
                    a0, a1 = g * G, min((g + 1) * G, NUMANGLE)
                    asub = sub[a0:a1]
                    rlo = int(asub.min()) // B
                    nch = int(asub.max()) // B - rlo + 1
                    d = np.full((G, PX), PAD_D, np.int16)
                    d[:a1 - a0] = (asub - rlo * B).astype(np.int16)
                    tiles.append((nch, bxi, g, rlo, d))
            tiles.sort(key=lambda t: t[0])
            nchs = [t[0] for t in tiles]
            smax = nchs[-1]
            planes = []
            off = 0
            for n in range(smax):
                fn = next(i for i, v in enumerate(nchs) if v > n)
                planes.append((fn, off))
                off += TPS - fn
            emax = max(emax, off)
            nch_max = max(nch_max, smax)
            blocks = [[None] * NG for _ in range(XGRP)]
            for ti, (nch, bxi, g, rlo, d) in enumerate(tiles):
                ecols = [planes[n][1] + (ti - planes[n][0])
                         for n in range(nch)]
                blocks[bxi][g] = (g, rlo, nch, ecols)
                d_parts.append(d)
            slabs.append({"planes": planes, "blocks": blocks})
    d_stream = np.ascontiguousarray(
        np.concatenate(d_parts, axis=1))  # [G, NSLAB*TPS*PX]
    assert d_stream.shape == (G, NSLAB * TPS * PX)
    return slabs, d_stream, nch_max, emax


def _pack_h(h_core: np.ndarray) -> np.ndarray:
    """[C, A, RHO] fp32 -> rectangle layout [128, NG*NR*128] bf16.

    Hp[ai*B+rj, ((g*NR)+r)*128 + c] = h[c, g*G+ai, r*B+rj]
    """
    hp = np.zeros((C, APAD, RPAD), np.float32)
    hp[:, :NUMANGLE, :NUMRHO] = h_core
    hp = hp.reshape(C, NG, G, NR, B)
    hp = hp.transpose(2, 4, 1, 3, 0)           # [G, B, NG, NR, C]
    full = np.zeros((128, NG * NR * C), BF16)
    full[:K] = hp.reshape(K, NG * NR * C).astype(BF16)
    return full


_SCHED_CACHE = None


def _schedule():
    global _SCHED_CACHE
    if _SCHED_CACHE is None:
        _SCHED_CACHE = _build_schedule()
    return _SCHED_CACHE


def _rj_col() -> np.ndarray:
    return (np.arange(128, dtype=np.float32) % B).reshape(128, 1)[:K]


def prepare_inputs(hough_feat: np.ndarray, mode: str = "full") -> list[dict]:
    _, d_stream, _, emax = _schedule()
    rj = _rj_col()
    extra = {}
    if mode == "pe":
        extra["e0"] = np.zeros((K, emax * PX), BF16)
    return [{"hp": _pack_h(hough_feat[i].astype(np.float32)),
             "d": d_stream, "rj": rj, **extra} for i in range(N)]


def build_bass(reps: int = 1, mode: str = "full"):
    """Build the Bass program (single-core SPMD; same program on all cores).

    mode: "full" | "pe" (skip E-gen + D DMA; E garbage — timing ablation)
        | "dve" (skip matmul/drain — timing ablation)
    """
    import concourse.mybir as mybir
    from concourse import bacc
    from concourse.tile import TileContext

    slabs, d_stream, nch_max, emax = _schedule()

    nc = bacc.Bacc(None, target_bir_lowering=False)
    hp_d = nc.dram_tensor("hp", [128, NG * NR * C], mybir.dt.bfloat16,
                          kind="ExternalInput")
    d_d = rj_d = e0_d = None
    if mode in ("full", "dve", "dvec", "ddma"):
        d_d = nc.dram_tensor("d", [G, NSLAB * TPS * PX], mybir.dt.int16,
                             kind="ExternalInput")
        rj_d = nc.dram_tensor("rj", [K, 1], mybir.dt.float32,
                              kind="ExternalInput")
    if mode == "pe":
        e0_d = nc.dram_tensor("e0", [128, emax * PX], mybir.dt.bfloat16,
                              kind="ExternalInput")
    out_d = nc.dram_tensor("out", [128, HIMG * WIMG], mybir.dt.float32,
                           kind="ExternalOutput")

    with TileContext(nc) as tc:
        with tc.tile_pool(name="hp_pool", bufs=1) as hp_pool, \
             tc.tile_pool(name="const_pool", bufs=1) as const_pool, \
             tc.tile_pool(name="d_pool", bufs=4) as d_pool, \
             tc.tile_pool(name="e_pool", bufs=3) as e_pool, \
             tc.tile_pool(name="stage_pool", bufs=3) as stage_pool, \
             tc.tile_pool(name="psum_pool", bufs=8, space="PSUM") as psum_pool:
            hp_t = hp_pool.tile([128, NG * NR * C], mybir.dt.bfloat16)
            nc.sync.dma_start(hp_t[:], hp_d[:])
            rj_t = e0_t = dc_t = None
            if mode in ("full", "dve", "dvec", "ddma"):
                rj_t = const_pool.tile([K, 1], mybir.dt.float32)
                nc.sync.dma_start(rj_t[:], rj_d[:])
            if mode == "dvec":
                dc_t = const_pool.tile([K, TPS * PX], mybir.dt.int16)
                nc.sync.dma_start(
                    dc_t[:],
                    d_d[:, :TPS * PX].unsqueeze(1)
                    .broadcast_to((G, B, TPS * PX)))
            if mode == "pe":
                e0_t = const_pool.tile([K, emax * PX], mybir.dt.bfloat16)
                nc.sync.dma_start(e0_t[:], e0_d[:])

            LOOKAHEAD = 3

            def issue_d(sj):
                """Start the (broadcast-replicating) D DMA for slab sj."""
                t = d_pool.tile([K, TPS * PX], mybir.dt.int16, tag="d")
                dma_eng = nc.sync if sj % 2 == 0 else nc.scalar
                dma_eng.dma_start(
                    t[:],
                    d_d[:, sj * TPS * PX:(sj + 1) * TPS * PX]
                    .unsqueeze(1)
                    .broadcast_to((G, B, TPS * PX)))
                return t

            for _ in range(reps):
                si = 0
                d_tiles = {}
                if mode in ("full", "dve", "ddma"):
                    for sj in range(LOOKAHEAD):
                        d_tiles[sj] = issue_d(sj)
                for by in range(NBY):
                    for bxg in range(NBX // XGRP):
                        slab = slabs[si]
                        if mode in ("full", "dve", "dvec", "ddma"):
                            if mode != "dvec":
                                if si + LOOKAHEAD < NSLAB:
                                    d_tiles[si + LOOKAHEAD] = issue_d(
                                        si + LOOKAHEAD)
                                dt_ = d_tiles.pop(si)
                            else:
                                dt_ = dc_t
                            et = e_pool.tile([K, emax * PX],
                                             mybir.dt.bfloat16, tag="e")
                            for n, (fn, e0) in enumerate(
                                    slab["planes"] if mode != "ddma" else []):
                                w = (TPS - fn) * PX
                                nc.vector.tensor_scalar(
                                    et[:, e0 * PX:e0 * PX + w],
                                    dt_[:, fn * PX:fn * PX + w],
                                    B * n,
                                    rj_t[:, 0:1],
                                    op0=mybir.AluOpType.subtract,
                                    op1=mybir.AluOpType.is_equal)
                        else:  # pe
                            et = e0_t
                        stage = stage_pool.tile([128, XGRP * PX],
                                                mybir.dt.float32, tag="stage")
                        for bxi in (range(XGRP) if mode not in
                                    ("dve", "dvec", "ddma") else []):
                            ps = psum_pool.tile([128, PX], mybir.dt.float32,
                                                tag="ps")
                            chunks = []
                            for (g, rlo, nch, ecols) in slab["blocks"][bxi]:
                                for k in range(nch):
                                    chunks.append(((g * NR + rlo + k) * C,
                                                   ecols[k] * PX))
                            nch_tot = len(chunks)
                            for ci, (col, eoff) in enumerate(chunks):
                                nc.tensor.matmul(
                                    ps[:],
                                    hp_t[:K, col:col + C],
                                    et[:, eoff:eoff + PX],
                                    start=(ci == 0),
                                    stop=(ci == nch_tot - 1),
                                )
                            nc.scalar.copy(
                                stage[:].rearrange(
                                    "p (dy bxs dx) -> p dy bxs dx",
                                    dy=BH, bxs=XGRP)[:, :, bxi, :],
                                ps[:].rearrange("p (dy dx) -> p dy dx", dy=BH),
                            )
                        src = (stage[:] if mode not in
                               ("dve", "dvec", "ddma")
                               else hp_t[:, :2 * XGRP * PX]
                               .bitcast(mybir.dt.float32))
                        # slab-major flat output (contiguous 2KB runs);
                        # host un-scrambles.  Opposite queue parity from D.
                        out_eng = nc.scalar if si % 2 == 0 else nc.sync
                        out_eng.dma_start(
                            out_d[:, si * XGRP * PX:(si + 1) * XGRP * PX],
                            src,
                        )
                        si += 1
    nc.compile()
    return nc


def _run(nc, in_maps, n_cores):
    from concourse.bass_utils import run_bass_kernel_spmd
    return run_bass_kernel_spmd(nc, in_maps, core_ids=list(range(n_cores)))


def _unscramble(flat: np.ndarray) -> np.ndarray:
    """Slab-major device layout -> [C, H, W].

    flat[c, ((si*BH + dy)*XGRP + bxi)*BW + dx], si = by*(NBX//XGRP) + bxg.
    """
    a = flat.reshape(C, NBY, NBX // XGRP, BH, XGRP, BW)
    return np.ascontiguousarray(
        a.transpose(0, 1, 3, 2, 4, 5).reshape(C, HIMG, WIMG))


def kernel(hough_feat: np.ndarray) -> np.ndarray:
    hough_feat = np.asarray(hough_feat)
    assert hough_feat.shape == (N, C, NUMANGLE, NUMRHO)
    nc = build_bass(reps=1)
    in_maps = prepare_inputs(hough_feat)
    res = _run(nc, in_maps, N)
    out = np.stack([_unscramble(r["out"]) for r in res.results])
    return out.astype(hough_feat.dtype, copy=False)


if __name__ == "__main__":
    slabs, d_stream, nch_max, emax = _schedule()
    tot = sum(t[2] for s in slabs for blk in s["blocks"] for t in blk)
    npl = sum(len(s["planes"]) for s in slabs)
    print(f"chunks total={tot} nch_max={nch_max} emax={emax} "
          f"plane-ops={npl} D MB={d_stream.nbytes/1e6:.1f}")
